# revision 7
# baseline (speedup 1.0000x reference)
"""Trainium2 Bass kernel for nn_ContrastLoss (LayerNorm + label segment-sum +
EMA codebook contrast loss), data-parallel over 8 NeuronCores.

Contract: kernel(**inputs) takes the FULL unsharded inputs
  input_f [128,1024,768] f32, char_dic [96,768] f32, ln_w [768] f32,
  ln_b [768] f32, target [128,1024] int64
and returns the full output (f32 scalar), matching reference.reference.

v2 design (hardcoded for the shapes above):
 - shard the batch dim over 8 cores: 16 batches = 16384 tokens per core
 - token->partition layout: partition p owns tokens p*128..p*128+127, so
   the labels land in SBUF with ONE contiguous [128,128] DMA (512B rows)
 - per core, stream 16 tiles of [128 part x 8 tok x 768] f32 on 4 DMA
   queues (sync/tensor/scalar/gpsimd; 2 tokens each)
 - per tile:
     * ACT: 8x cast f32->bf16 with accum_out => per-token row sums
     * sumsq split across engines: DVE scalar_tensor_tensor (x*x accum),
       last tokens optionally on ACT (Square+accum) / GpSimd
     * DVE: 768*var = sumsq - sums^2/768 (2 small batched ops)
     * ACT: rstd = Rsqrt(var/768+eps) [128,8]; DVE writes the bf16 std
       column (reciprocal of rstd) at col 768 (counts come from the
       matmul: sum_tok rstd*std = n_label); col 769 zero pad
     * DVE one-hot: tensor_scalar (iota == lab) * rstd, bf16 [128,128]
       (padded to 128 labels so LDWEIGHTS takes the fast path)
     * PE: 2 matmuls per token accumulate PSUM [128,384]+[128,386]
   (ln_w/ln_b fold out of the streaming loop: the -mu*rstd shift is
    recovered in the tail from the row-mean of the scaled segment sums)
 - bf16 AllReduce of the [96,769] partial (sums | counts) across cores
 - tail math (group sums, positive term, EMA update, LayerNorm, negative
   term) computed replicated on every core; host reads core 0's scalar
"""

import os
import sys

for _p in ("/opt/trn_rl_repo",):
    if _p not in sys.path:
        sys.path.insert(0, _p)

import numpy as np
import ml_dtypes

import concourse.bass as bass
import concourse.bacc as bacc
import concourse.tile as tile
from concourse import mybir
from concourse.bass_utils import run_bass_kernel_spmd

F32 = mybir.dt.float32
BF16 = mybir.dt.bfloat16
AF = mybir.ActivationFunctionType
OP = mybir.AluOpType

N_CORES = 8
B, S, D = 128, 1024, 768
NCHAR = 96
NPAD = 128                                 # one-hot padded to 128 labels
EPS = 1e-5
EMA = 0.1

TOK_PER_CORE = (B // N_CORES) * S          # 16384
T = 8                                      # tokens per partition per tile
TILE_TOK = 128 * T                         # 1024 tokens per tile
N_TILES = TOK_PER_CORE // TILE_TOK         # 16
W = D + 2                                  # token row: 768 x | std | pad

# tuning knobs
SQ_ACT = int(os.environ.get("K_SQ_ACT", "1"))   # sumsq tokens on ACT
SQ_GPS = int(os.environ.get("K_SQ_GPS", "0"))   # sumsq tokens on GpSimd (Pool
                                                # rejects TensorScalarPtr; keep 0)
XP_BUFS = int(os.environ.get("K_XP_BUFS", "4"))
SAFE_CC = os.environ.get("K_SAFE_CC", "0") == "1"


def build_kernel(n_tiles=N_TILES, trivial_wb=False):
    tok_per_core = n_tiles * TILE_TOK
    nc = bacc.Bacc("TRN2", target_bir_lowering=False, debug=False,
                   num_devices=N_CORES)

    x_d = nc.dram_tensor("x", [tok_per_core, D], F32, kind="ExternalInput")
    lab_d = nc.dram_tensor("lab", [tok_per_core], F32, kind="ExternalInput")
    char_d = nc.dram_tensor("char", [NCHAR, D], F32, kind="ExternalInput")
    wbc_d = nc.dram_tensor("wbc", [NCHAR, D], F32, kind="ExternalInput")
    bbc_d = nc.dram_tensor("bbc", [NCHAR, D], F32, kind="ExternalInput")
    out_d = nc.dram_tensor("out", [1, 1], F32, kind="ExternalOutput")

    # constants embedded in the NEFF
    iota_np = np.tile(np.arange(NPAD, dtype=np.float32), (128, 1))
    iota_d = nc.inline_tensor(iota_np.astype(ml_dtypes.bfloat16),
                              name="iota128")
    mask_np = np.ones((NCHAR, 1), dtype=np.float32)
    mask_np[0, 0] = 0.0
    mask_d = nc.inline_tensor(mask_np, name="maskrow")
    ones96_d = nc.inline_tensor(np.ones((NCHAR, 1), dtype=np.float32),
                                name="ones96")

    # collective bounce buffers (bf16 payload: scaled sums | counts)
    CCDT = F32 if SAFE_CC else BF16
    cc_in = nc.dram_tensor("cc_in", [NCHAR, D + 1], CCDT)
    cc_out = nc.dram_tensor("cc_out", [NCHAR, D + 1], CCDT,
                            addr_space="Shared")

    # partition p owns tokens p*(8*n_tiles) .. p*(8*n_tiles)+8*n_tiles-1
    x_r = x_d.ap().rearrange("(p t f) d -> t p f d",
                             p=128, t=n_tiles, f=T)
    lab_r = lab_d.ap().rearrange("(p i) -> p i", p=128, i=n_tiles * T)

    n_dve = T - SQ_ACT - SQ_GPS            # sumsq tokens on DVE

    with tile.TileContext(nc) as tc:
        with (
            tc.tile_pool(name="consts", bufs=1) as consts,
            tc.tile_pool(name="xp", bufs=XP_BUFS) as xp,
            tc.tile_pool(name="xbp", bufs=3) as xbp,
            tc.tile_pool(name="stp", bufs=3) as stp,
            tc.tile_pool(name="ohp", bufs=3) as ohp,
            tc.tile_pool(name="tailp", bufs=1) as tailp,
            tc.tile_pool(name="psum", bufs=1, space="PSUM") as psp,
        ):
            # --- constants (scalar queue; x streaming uses the others) ---
            iota_sb = consts.tile([128, NPAD], BF16)
            nc.scalar.dma_start(out=iota_sb[:], in_=iota_d.ap())
            lab_sb = consts.tile([128, n_tiles * T], F32)
            nc.scalar.dma_start(out=lab_sb[:], in_=lab_r)
            eps128 = consts.tile([128, 1], F32)
            nc.vector.memset(eps128[:], EPS)
            eps96 = consts.tile([NCHAR, 1], F32)
            nc.vector.memset(eps96[:], EPS)
            mask_sb = consts.tile([NCHAR, 1], F32)
            nc.scalar.dma_start(out=mask_sb[:], in_=mask_d.ap())
            ones96_sb = consts.tile([NCHAR, 1], F32)
            nc.scalar.dma_start(out=ones96_sb[:], in_=ones96_d.ap())
            char_sb = consts.tile([NCHAR, D], F32)
            nc.scalar.dma_start(out=char_sb[:], in_=char_d.ap())
            if not trivial_wb:
                wbc_sb = consts.tile([NCHAR, D], F32)
                nc.scalar.dma_start(out=wbc_sb[:], in_=wbc_d.ap())
                bbc_sb = consts.tile([NCHAR, D], F32)
                nc.scalar.dma_start(out=bbc_sb[:], in_=bbc_d.ap())
            # per-engine scratch sinks for the sumsq main output
            trash_v = consts.tile([128, D], BF16)
            trash_a = consts.tile([128, D], BF16)
            trash_g = consts.tile([128, D], BF16)

            # --- PSUM accumulators for the streaming segment-sum ---
            psA = psp.tile([NPAD, 384], F32)
            psB = psp.tile([NPAD, 386], F32)

            # --- streaming loop (stats->onehot->matmul lag one tile) ---
            # DMA-capable queues: sync, scalar (ACT), gpsimd
            dma_chunks = [(nc.sync, 0, 4), (nc.scalar, 4, 6), (nc.gpsimd, 6, 8)]
            state = {}

            def phase_a(i):
                x_t = xp.tile([128, T, D], F32)
                for q, lo, hi in dma_chunks:
                    q.dma_start(out=x_t[:, lo:hi, :],
                                in_=x_r[i, :, lo:hi, :])

                xb_t = xbp.tile([128, T, W], BF16)
                sums = stp.tile([128, T], F32)
                sumsq = stp.tile([128, T], F32)
                # ACT: cast to bf16 + row sums via accumulator
                for t in range(T):
                    nc.scalar.activation(
                        xb_t[:, t, 0:D], x_t[:, t, :], AF.Identity,
                        accum_out=sums[:, t:t + 1])
                # sumsq = row sums of squares (from the bf16 copy)
                for t in range(n_dve):
                    nc.vector.scalar_tensor_tensor(
                        trash_v[:], xb_t[:, t, 0:D], 1.0,
                        xb_t[:, t, 0:D], OP.mult, OP.mult,
                        accum_out=sumsq[:, t:t + 1])
                for k in range(SQ_GPS):
                    t = n_dve + k
                    nc.gpsimd.scalar_tensor_tensor(
                        trash_g[:], xb_t[:, t, 0:D], 1.0,
                        xb_t[:, t, 0:D], OP.mult, OP.mult,
                        accum_out=sumsq[:, t:t + 1])
                for k in range(SQ_ACT):
                    t = n_dve + SQ_GPS + k
                    nc.scalar.activation(
                        trash_a[:], xb_t[:, t, 0:D], AF.Square,
                        accum_out=sumsq[:, t:t + 1])
                # 768*var = sumsq - sums^2/768 (batched [128,8])
                s2 = stp.tile([128, T], F32)
                nc.vector.scalar_tensor_tensor(
                    s2[:], sums[:], 1.0 / D, sums[:], OP.mult, OP.mult)
                xv = stp.tile([128, T], F32)
                nc.vector.tensor_sub(xv[:], sumsq[:], s2[:])
                nc.vector.memset(xb_t[:, :, D + 1], 0.0)
                state[i] = (x_t, xb_t, xv)

            def phase_b(i):
                x_t, xb_t, xv = state.pop(i)
                # std column (bf16) on ACT; rstd via DVE reciprocal
                nc.scalar.activation(xb_t[:, :, D], xv[:], AF.Sqrt,
                                     bias=eps128[:], scale=1.0 / D)
                rstd = stp.tile([128, T], F32)
                nc.vector.reciprocal(rstd[:], xb_t[:, :, D])

                # scaled one-hot: (iota == label) * rstd, bf16 [128,128]
                oh_t = ohp.tile([128, T, NPAD], BF16)
                base = i * T
                for t in range(T):
                    nc.vector.tensor_scalar(
                        oh_t[:, t, :], iota_sb[:],
                        lab_sb[:, base + t:base + t + 1],
                        rstd[:, t:t + 1], OP.is_equal, OP.mult)

                first, last = i == 0, i == n_tiles - 1
                for t in range(T):
                    st0 = first and t == 0
                    sp0 = last and t == T - 1
                    nc.tensor.matmul(psA[:], oh_t[:, t, :],
                                     xb_t[:, t, 0:384], start=st0, stop=sp0)
                    nc.tensor.matmul(psB[:], oh_t[:, t, :],
                                     xb_t[:, t, 384:W], start=st0, stop=sp0)

            for i in range(n_tiles + 1):
                if i < n_tiles:
                    phase_a(i)
                if i >= 1:
                    phase_b(i - 1)

            # --- local partials -> bf16 -> DRAM -> AllReduce ---
            # psA cols = feat 0..383; psB cols 0..383 = feat 384..767,
            # col 384 = counts (sum rstd*std), col 385 = pad junk
            acc = tailp.tile([NCHAR, D + 1], CCDT)
            nc.vector.tensor_copy(acc[:, 0:384], psA[0:NCHAR, :])
            nc.vector.tensor_copy(acc[:, 384:D + 1], psB[0:NCHAR, 0:385])
            nc.sync.dma_start(out=cc_in.ap(), in_=acc[:])
            nc.gpsimd.collective_compute(
                "AllReduce", OP.add,
                replica_groups=[list(range(N_CORES))],
                ins=[cc_in.ap()], outs=[cc_out.ap()],
            )
            red = tailp.tile([NCHAR, D + 1], CCDT)
            nc.sync.dma_start(out=red[:], in_=cc_out.ap())
            cnt = tailp.tile([NCHAR, 1], F32)
            nc.vector.tensor_copy(cnt[:], red[:, D:D + 1])

            # beta_i = mean_d S[i, d]  (the LayerNorm -mu*rstd correction
            # folds into a row-mean of the scaled segment sums)
            rs = tailp.tile([NCHAR, 1], F32)
            nc.vector.reduce_sum(rs[:], red[:, 0:D],
                                 axis=mybir.AxisListType.X)
            nb = tailp.tile([NCHAR, 1], F32)
            nc.vector.tensor_scalar(nb[:], rs[:], -1.0 / D, None, OP.mult)
            # group_sum = char + (S - beta)*w + counts*b
            group = tailp.tile([NCHAR, D], F32)
            if trivial_wb:
                nc.vector.scalar_tensor_tensor(group[:], red[:, 0:D], nb[:],
                                               char_sb[:], OP.add, OP.add)
            else:
                tmp1 = tailp.tile([NCHAR, D], F32)
                nc.vector.scalar_tensor_tensor(tmp1[:], bbc_sb[:], cnt[:],
                                               char_sb[:], OP.mult, OP.add)
                nc.vector.scalar_tensor_tensor(group[:], red[:, 0:D], nb[:],
                                               wbc_sb[:], OP.add, OP.mult)
                nc.vector.tensor_add(group[:], group[:], tmp1[:])

            # positive = sum(group^2) (divide by D at the very end)
            sq = tailp.tile([NCHAR, D], F32)
            pos_col = tailp.tile([NCHAR, 1], F32)
            nc.scalar.activation(sq[:], group[:], AF.Square,
                                 accum_out=pos_col[:])
            pos_ps = psp.tile([1, 1], F32)
            nc.tensor.matmul(pos_ps[:], ones96_sb[:], pos_col[:],
                             start=True, stop=True)
            pos_sb = tailp.tile([1, 1], F32)
            nc.vector.tensor_copy(pos_sb[:], pos_ps[:])

            # EMA update: new_char = char + 0.1 * group/(counts+1);
            # row 0 kept via a zeroed scale factor
            cnt1 = tailp.tile([NCHAR, 1], F32)
            nc.vector.tensor_scalar(cnt1[:], cnt[:], 1.0, None, OP.add)
            invc = tailp.tile([NCHAR, 1], F32)
            nc.vector.reciprocal(invc[:], cnt1[:])
            invcE = tailp.tile([NCHAR, 1], F32)
            nc.vector.tensor_scalar(invcE[:], invc[:], EMA, None, OP.mult)
            nc.vector.memset(invcE[0:1, :], 0.0)
            newc = tailp.tile([NCHAR, D], F32)
            nc.vector.scalar_tensor_tensor(newc[:], group[:], invcE[:],
                                           char_sb[:], OP.mult, OP.add)

            # LayerNorm(new_char) with w/b
            bn2 = tailp.tile([NCHAR, 2, 6], F32)
            for g in range(2):
                nc.vector.bn_stats(bn2[:, g, :], newc[:, g * 384:(g + 1) * 384])
            st2 = tailp.tile([NCHAR, 2], F32)
            nc.vector.bn_aggr(st2[:], bn2[:])
            std2 = tailp.tile([NCHAR, 1], F32)
            nc.scalar.activation(std2[:], st2[:, 1:2], AF.Sqrt,
                                 bias=eps96[:], scale=1.0)
            rstd2 = tailp.tile([NCHAR, 1], F32)
            nc.vector.reciprocal(rstd2[:], std2[:])
            nmr2 = tailp.tile([NCHAR, 1], F32)
            nc.vector.scalar_tensor_tensor(nmr2[:], st2[:, 0:1], -1.0,
                                           rstd2[:], OP.mult, OP.mult)
            nrm = tailp.tile([NCHAR, D], F32)
            nc.scalar.activation(nrm[:], newc[:], AF.Identity,
                                 bias=nmr2[:], scale=rstd2[:])
            if trivial_wb:
                fin = nrm
            else:
                fin = tailp.tile([NCHAR, D], F32)
                nc.vector.tensor_mul(fin[:], nrm[:], wbc_sb[:])
                nc.vector.tensor_add(fin[:], fin[:], bbc_sb[:])

            # s = sum over rows 1..95 -> [1,768]; negative = sum(s^2)
            sA = psp.tile([1, 384], F32)
            sB = psp.tile([1, 384], F32)
            nc.tensor.matmul(sA[:], mask_sb[:], fin[:, 0:384],
                             start=True, stop=True)
            nc.tensor.matmul(sB[:], mask_sb[:], fin[:, 384:D],
                             start=True, stop=True)
            sqA = tailp.tile([1, 384], F32)
            sqB = tailp.tile([1, 384], F32)
            negA = tailp.tile([1, 1], F32)
            negB = tailp.tile([1, 1], F32)
            nc.scalar.activation(sqA[:], sA[:], AF.Square, accum_out=negA[:])
            nc.scalar.activation(sqB[:], sB[:], AF.Square, accum_out=negB[:])

            res = tailp.tile([1, 1], F32)
            nc.vector.tensor_add(res[:], negA[:], negB[:])
            nc.vector.tensor_sub(res[:], res[:], pos_sb[:])
            nc.vector.tensor_scalar(res[:], res[:], 1.0 / D, None, OP.mult)
            nc.sync.dma_start(out=out_d.ap(), in_=res[:])

    nc.finalize()
    return nc


_NC_CACHE = {}


def _get_nc(trivial_wb):
    if trivial_wb not in _NC_CACHE:
        _NC_CACHE[trivial_wb] = build_kernel(trivial_wb=trivial_wb)
    return _NC_CACHE[trivial_wb]


def make_in_maps(input_f, char_dic, ln_w, ln_b, target):
    input_f = np.ascontiguousarray(np.asarray(input_f, dtype=np.float32))
    char_dic = np.ascontiguousarray(np.asarray(char_dic, dtype=np.float32))
    ln_w = np.asarray(ln_w, dtype=np.float32)
    ln_b = np.asarray(ln_b, dtype=np.float32)
    labels = np.asarray(target).reshape(B, S).astype(np.float32)

    wbc = np.ascontiguousarray(np.broadcast_to(ln_w[None, :], (NCHAR, D)))
    bbc = np.ascontiguousarray(np.broadcast_to(ln_b[None, :], (NCHAR, D)))

    bpc = B // N_CORES
    in_maps = []
    for c in range(N_CORES):
        x_c = input_f[c * bpc:(c + 1) * bpc].reshape(TOK_PER_CORE, D)
        l_c = labels[c * bpc:(c + 1) * bpc].reshape(TOK_PER_CORE)
        in_maps.append({
            "x": np.ascontiguousarray(x_c),
            "lab": np.ascontiguousarray(l_c),
            "char": char_dic,
            "wbc": wbc,
            "bbc": bbc,
        })
    return in_maps


def run(trace=False, **inputs):
    trivial_wb = bool(
        np.all(np.asarray(inputs["ln_w"], dtype=np.float32) == 1.0)
        and np.all(np.asarray(inputs["ln_b"], dtype=np.float32) == 0.0))
    nc = _get_nc(trivial_wb)
    in_maps = make_in_maps(**inputs)
    res = run_bass_kernel_spmd(nc, in_maps, core_ids=list(range(N_CORES)),
                               trace=trace)
    out = np.float32(res.results[0]["out"][0, 0])
    return out, res


def kernel(**inputs):
    out, _ = run(trace=False, **inputs)
    return np.array(out, dtype=np.float32)


if __name__ == "__main__":
    np.random.seed(0)
    input_f = np.random.randn(B, S, D).astype(np.float32)
    char_dic = np.random.randn(NCHAR, D).astype(np.float32)
    ln_w = np.ones(D, np.float32)
    ln_b = np.zeros(D, np.float32)
    target = np.random.randint(0, NCHAR, (B, S)).astype(np.int64)
    out = kernel(input_f=input_f, char_dic=char_dic, ln_w=ln_w,
                 ln_b=ln_b, target=target)
    print("kernel out:", out)


# revision 9
# speedup vs baseline: 1.0365x; 1.0365x over previous
"""Trainium2 Bass kernel for nn_ContrastLoss (LayerNorm + label segment-sum +
EMA codebook contrast loss), data-parallel over 8 NeuronCores.

Contract: kernel(**inputs) takes the FULL unsharded inputs
  input_f [128,1024,768] f32, char_dic [96,768] f32, ln_w [768] f32,
  ln_b [768] f32, target [128,1024] int64
and returns the full output (f32 scalar), matching reference.reference.

v2 design (hardcoded for the shapes above):
 - shard the batch dim over 8 cores: 16 batches = 16384 tokens per core
 - token->partition layout: partition p owns tokens p*128..p*128+127, so
   the labels land in SBUF with ONE contiguous [128,128] DMA (512B rows)
 - per core, stream 16 tiles of [128 part x 8 tok x 768] f32 on 4 DMA
   queues (sync/tensor/scalar/gpsimd; 2 tokens each)
 - per tile:
     * ACT: 8x cast f32->bf16 with accum_out => per-token row sums
     * sumsq split across engines: DVE scalar_tensor_tensor (x*x accum),
       last tokens optionally on ACT (Square+accum) / GpSimd
     * DVE: 768*var = sumsq - sums^2/768 (2 small batched ops)
     * ACT: rstd = Rsqrt(var/768+eps) [128,8]; DVE writes the bf16 std
       column (reciprocal of rstd) at col 768 (counts come from the
       matmul: sum_tok rstd*std = n_label); col 769 zero pad
     * DVE one-hot: tensor_scalar (iota == lab) * rstd, bf16 [128,128]
       (padded to 128 labels so LDWEIGHTS takes the fast path)
     * PE: 2 matmuls per token accumulate PSUM [128,384]+[128,386]
   (ln_w/ln_b fold out of the streaming loop: the -mu*rstd shift is
    recovered in the tail from the row-mean of the scaled segment sums)
 - bf16 AllReduce of the [96,769] partial (sums | counts) across cores
 - tail math (group sums, positive term, EMA update, LayerNorm, negative
   term) computed replicated on every core; host reads core 0's scalar
"""

import os
import sys

for _p in ("/opt/trn_rl_repo",):
    if _p not in sys.path:
        sys.path.insert(0, _p)

import numpy as np
import ml_dtypes

import concourse.bass as bass
import concourse.bacc as bacc
import concourse.tile as tile
from concourse import mybir
from concourse.bass_utils import run_bass_kernel_spmd

F32 = mybir.dt.float32
BF16 = mybir.dt.bfloat16
AF = mybir.ActivationFunctionType
OP = mybir.AluOpType

N_CORES = 8
B, S, D = 128, 1024, 768
NCHAR = 96
NPAD = 128                                 # one-hot padded to 128 labels
EPS = 1e-5
EMA = 0.1

TOK_PER_CORE = (B // N_CORES) * S          # 16384
T = 8                                      # tokens per partition per tile
TILE_TOK = 128 * T                         # 1024 tokens per tile
N_TILES = TOK_PER_CORE // TILE_TOK         # 16
W = D + 2                                  # token row: 768 x | std | pad

# tuning knobs
SQ_ACT = int(os.environ.get("K_SQ_ACT", "1"))   # sumsq tokens on ACT
SQ_GPS = int(os.environ.get("K_SQ_GPS", "0"))   # sumsq tokens on GpSimd (Pool
                                                # rejects TensorScalarPtr; keep 0)
XP_BUFS = int(os.environ.get("K_XP_BUFS", "4"))
SAFE_CC = os.environ.get("K_SAFE_CC", "0") == "1"


def build_kernel(n_tiles=N_TILES, trivial_wb=False):
    tok_per_core = n_tiles * TILE_TOK
    nc = bacc.Bacc("TRN2", target_bir_lowering=False, debug=False,
                   num_devices=N_CORES)

    x_d = nc.dram_tensor("x", [tok_per_core, D], F32, kind="ExternalInput")
    lab_d = nc.dram_tensor("lab", [tok_per_core], F32, kind="ExternalInput")
    char_d = nc.dram_tensor("char", [NCHAR, D], F32, kind="ExternalInput")
    wbc_d = nc.dram_tensor("wbc", [NCHAR, D], F32, kind="ExternalInput")
    bbc_d = nc.dram_tensor("bbc", [NCHAR, D], F32, kind="ExternalInput")
    out_d = nc.dram_tensor("out", [1, 1], F32, kind="ExternalOutput")

    # constants embedded in the NEFF
    iota_np = np.tile(np.arange(NPAD, dtype=np.float32), (128, 1))
    iota_d = nc.inline_tensor(iota_np.astype(ml_dtypes.bfloat16),
                              name="iota128")
    mask_np = np.ones((NCHAR, 1), dtype=np.float32)
    mask_np[0, 0] = 0.0
    mask_d = nc.inline_tensor(mask_np, name="maskrow")
    ones96_d = nc.inline_tensor(np.ones((NCHAR, 1), dtype=np.float32),
                                name="ones96")

    # collective bounce buffers (bf16 payload: scaled sums | counts)
    CCDT = F32 if SAFE_CC else BF16
    cc_in = nc.dram_tensor("cc_in", [NCHAR, D + 1], CCDT)
    cc_out = nc.dram_tensor("cc_out", [NCHAR, D + 1], CCDT,
                            addr_space="Shared")

    # partition p owns tokens p*(8*n_tiles) .. p*(8*n_tiles)+8*n_tiles-1
    x_r = x_d.ap().rearrange("(p t f) d -> t p f d",
                             p=128, t=n_tiles, f=T)
    lab_r = lab_d.ap().rearrange("(p i) -> p i", p=128, i=n_tiles * T)

    n_dve = T - SQ_ACT - SQ_GPS            # sumsq tokens on DVE

    with tile.TileContext(nc) as tc:
        with (
            tc.tile_pool(name="consts", bufs=1) as consts,
            tc.tile_pool(name="xp", bufs=XP_BUFS) as xp,
            tc.tile_pool(name="xbp", bufs=3) as xbp,
            tc.tile_pool(name="stp", bufs=3) as stp,
            tc.tile_pool(name="ohp", bufs=3) as ohp,
            tc.tile_pool(name="tailp", bufs=1) as tailp,
            tc.tile_pool(name="psum", bufs=1, space="PSUM") as psp,
        ):
            # --- constants (scalar queue; x streaming uses the others) ---
            iota_sb = consts.tile([128, NPAD], BF16)
            nc.scalar.dma_start(out=iota_sb[:], in_=iota_d.ap())
            lab_sb = consts.tile([128, n_tiles * T], F32)
            nc.scalar.dma_start(out=lab_sb[:], in_=lab_r)
            eps128 = consts.tile([128, 1], F32)
            nc.vector.memset(eps128[:], EPS)
            eps96 = consts.tile([NCHAR, 1], F32)
            nc.vector.memset(eps96[:], EPS)
            mask_sb = consts.tile([NCHAR, 1], F32)
            nc.scalar.dma_start(out=mask_sb[:], in_=mask_d.ap())
            ones96_sb = consts.tile([NCHAR, 1], F32)
            nc.scalar.dma_start(out=ones96_sb[:], in_=ones96_d.ap())
            char_sb = consts.tile([NCHAR, D], F32)
            nc.scalar.dma_start(out=char_sb[:], in_=char_d.ap())
            if not trivial_wb:
                wbc_sb = consts.tile([NCHAR, D], F32)
                nc.scalar.dma_start(out=wbc_sb[:], in_=wbc_d.ap())
                bbc_sb = consts.tile([NCHAR, D], F32)
                nc.scalar.dma_start(out=bbc_sb[:], in_=bbc_d.ap())
            # per-engine scratch sinks for the sumsq main output
            trash_v = consts.tile([128, D], BF16)
            trash_a = consts.tile([128, D], BF16)
            trash_g = consts.tile([128, D], BF16)

            # --- PSUM accumulators for the streaming segment-sum ---
            psA = psp.tile([NPAD, 384], F32)
            psB = psp.tile([NPAD, 386], F32)

            # --- streaming loop ---
            # DMA triggers run PF tiles ahead of the consuming compute so
            # the scalar-queue trigger (which sits in ACT's instruction
            # stream) keeps its prefetch lead; stats->onehot->matmul lag
            # one tile behind the cast/stats phase.
            # DMA-capable queues: sync, scalar (ACT), gpsimd
            dma_chunks = [(nc.sync, 0, 3), (nc.scalar, 3, 5), (nc.gpsimd, 5, 8)]
            PF = 2
            state = {}
            xq = {}

            def phase_dma(i):
                x_t = xp.tile([128, T, D], F32)
                for q, lo, hi in dma_chunks:
                    q.dma_start(out=x_t[:, lo:hi, :],
                                in_=x_r[i, :, lo:hi, :])
                xq[i] = x_t

            def phase_a(i):
                x_t = xq.pop(i)
                xb_t = xbp.tile([128, T, W], BF16)
                sums = stp.tile([128, T], F32)
                sumsq = stp.tile([128, T], F32)
                # ACT: cast to bf16 + row sums via accumulator
                for t in range(T):
                    nc.scalar.activation(
                        xb_t[:, t, 0:D], x_t[:, t, :], AF.Identity,
                        accum_out=sums[:, t:t + 1])
                # sumsq = row sums of squares (from the bf16 copy)
                for t in range(n_dve):
                    nc.vector.scalar_tensor_tensor(
                        trash_v[:], xb_t[:, t, 0:D], 1.0,
                        xb_t[:, t, 0:D], OP.mult, OP.mult,
                        accum_out=sumsq[:, t:t + 1])
                for k in range(SQ_GPS):
                    t = n_dve + k
                    nc.gpsimd.scalar_tensor_tensor(
                        trash_g[:], xb_t[:, t, 0:D], 1.0,
                        xb_t[:, t, 0:D], OP.mult, OP.mult,
                        accum_out=sumsq[:, t:t + 1])
                for k in range(SQ_ACT):
                    t = n_dve + SQ_GPS + k
                    nc.scalar.activation(
                        trash_a[:], xb_t[:, t, 0:D], AF.Square,
                        accum_out=sumsq[:, t:t + 1])
                # 768*var = sumsq - sums^2/768 (batched [128,8])
                s2 = stp.tile([128, T], F32)
                nc.vector.scalar_tensor_tensor(
                    s2[:], sums[:], 1.0 / D, sums[:], OP.mult, OP.mult)
                xv = stp.tile([128, T], F32)
                nc.vector.tensor_sub(xv[:], sumsq[:], s2[:])
                nc.vector.memset(xb_t[:, :, D + 1], 0.0)
                state[i] = (x_t, xb_t, xv)

            def phase_b(i):
                x_t, xb_t, xv = state.pop(i)
                # std column (bf16) on ACT; rstd via DVE reciprocal
                nc.scalar.activation(xb_t[:, :, D], xv[:], AF.Sqrt,
                                     bias=eps128[:], scale=1.0 / D)
                rstd = stp.tile([128, T], F32)
                nc.vector.reciprocal(rstd[:], xb_t[:, :, D])

                # scaled one-hot: (iota == label) * rstd, bf16 [128,128]
                oh_t = ohp.tile([128, T, NPAD], BF16)
                base = i * T
                for t in range(T):
                    nc.vector.tensor_scalar(
                        oh_t[:, t, :], iota_sb[:],
                        lab_sb[:, base + t:base + t + 1],
                        rstd[:, t:t + 1], OP.is_equal, OP.mult)

                first, last = i == 0, i == n_tiles - 1
                for t in range(T):
                    st0 = first and t == 0
                    sp0 = last and t == T - 1
                    nc.tensor.matmul(psA[:], oh_t[:, t, :],
                                     xb_t[:, t, 0:384], start=st0, stop=sp0)
                    nc.tensor.matmul(psB[:], oh_t[:, t, :],
                                     xb_t[:, t, 384:W], start=st0, stop=sp0)

            for i in range(PF):
                phase_dma(i)
            for i in range(n_tiles + 1):
                if i + PF < n_tiles:
                    phase_dma(i + PF)
                if i < n_tiles:
                    phase_a(i)
                if i >= 1:
                    phase_b(i - 1)

            # --- local partials -> bf16 -> DRAM -> AllReduce ---
            # psA cols = feat 0..383; psB cols 0..383 = feat 384..767,
            # col 384 = counts (sum rstd*std), col 385 = pad junk
            acc = tailp.tile([NCHAR, D + 1], CCDT)
            nc.vector.tensor_copy(acc[:, 0:384], psA[0:NCHAR, :])
            nc.vector.tensor_copy(acc[:, 384:D + 1], psB[0:NCHAR, 0:385])
            nc.sync.dma_start(out=cc_in.ap(), in_=acc[:])
            nc.gpsimd.collective_compute(
                "AllReduce", OP.add,
                replica_groups=[list(range(N_CORES))],
                ins=[cc_in.ap()], outs=[cc_out.ap()],
            )
            red = tailp.tile([NCHAR, D + 1], CCDT)
            nc.sync.dma_start(out=red[:], in_=cc_out.ap())
            cnt = tailp.tile([NCHAR, 1], F32)
            nc.vector.tensor_copy(cnt[:], red[:, D:D + 1])

            # beta_i = mean_d S[i, d]  (the LayerNorm -mu*rstd correction
            # folds into a row-mean of the scaled segment sums)
            rs = tailp.tile([NCHAR, 1], F32)
            nc.vector.reduce_sum(rs[:], red[:, 0:D],
                                 axis=mybir.AxisListType.X)
            nb = tailp.tile([NCHAR, 1], F32)
            nc.vector.tensor_scalar(nb[:], rs[:], -1.0 / D, None, OP.mult)
            # group_sum = char + (S - beta)*w + counts*b
            group = tailp.tile([NCHAR, D], F32)
            if trivial_wb:
                nc.vector.scalar_tensor_tensor(group[:], red[:, 0:D], nb[:],
                                               char_sb[:], OP.add, OP.add)
            else:
                tmp1 = tailp.tile([NCHAR, D], F32)
                nc.vector.scalar_tensor_tensor(tmp1[:], bbc_sb[:], cnt[:],
                                               char_sb[:], OP.mult, OP.add)
                nc.vector.scalar_tensor_tensor(group[:], red[:, 0:D], nb[:],
                                               wbc_sb[:], OP.add, OP.mult)
                nc.vector.tensor_add(group[:], group[:], tmp1[:])

            # positive = sum(group^2) (divide by D at the very end)
            sq = tailp.tile([NCHAR, D], F32)
            pos_col = tailp.tile([NCHAR, 1], F32)
            nc.scalar.activation(sq[:], group[:], AF.Square,
                                 accum_out=pos_col[:])
            pos_ps = psp.tile([1, 1], F32)
            nc.tensor.matmul(pos_ps[:], ones96_sb[:], pos_col[:],
                             start=True, stop=True)
            pos_sb = tailp.tile([1, 1], F32)
            nc.vector.tensor_copy(pos_sb[:], pos_ps[:])

            # EMA update: new_char = char + 0.1 * group/(counts+1);
            # row 0 kept via a zeroed scale factor
            cnt1 = tailp.tile([NCHAR, 1], F32)
            nc.vector.tensor_scalar(cnt1[:], cnt[:], 1.0, None, OP.add)
            invc = tailp.tile([NCHAR, 1], F32)
            nc.vector.reciprocal(invc[:], cnt1[:])
            invcE = tailp.tile([NCHAR, 1], F32)
            nc.vector.tensor_scalar(invcE[:], invc[:], EMA, None, OP.mult)
            nc.vector.memset(invcE[0:1, :], 0.0)
            newc = tailp.tile([NCHAR, D], F32)
            nc.vector.scalar_tensor_tensor(newc[:], group[:], invcE[:],
                                           char_sb[:], OP.mult, OP.add)

            # LayerNorm(new_char) with w/b
            bn2 = tailp.tile([NCHAR, 2, 6], F32)
            for g in range(2):
                nc.vector.bn_stats(bn2[:, g, :], newc[:, g * 384:(g + 1) * 384])
            st2 = tailp.tile([NCHAR, 2], F32)
            nc.vector.bn_aggr(st2[:], bn2[:])
            std2 = tailp.tile([NCHAR, 1], F32)
            nc.scalar.activation(std2[:], st2[:, 1:2], AF.Sqrt,
                                 bias=eps96[:], scale=1.0)
            rstd2 = tailp.tile([NCHAR, 1], F32)
            nc.vector.reciprocal(rstd2[:], std2[:])
            nmr2 = tailp.tile([NCHAR, 1], F32)
            nc.vector.scalar_tensor_tensor(nmr2[:], st2[:, 0:1], -1.0,
                                           rstd2[:], OP.mult, OP.mult)
            nrm = tailp.tile([NCHAR, D], F32)
            nc.scalar.activation(nrm[:], newc[:], AF.Identity,
                                 bias=nmr2[:], scale=rstd2[:])
            if trivial_wb:
                fin = nrm
            else:
                fin = tailp.tile([NCHAR, D], F32)
                nc.vector.tensor_mul(fin[:], nrm[:], wbc_sb[:])
                nc.vector.tensor_add(fin[:], fin[:], bbc_sb[:])

            # s = sum over rows 1..95 -> [1,768]; negative = sum(s^2)
            sA = psp.tile([1, 384], F32)
            sB = psp.tile([1, 384], F32)
            nc.tensor.matmul(sA[:], mask_sb[:], fin[:, 0:384],
                             start=True, stop=True)
            nc.tensor.matmul(sB[:], mask_sb[:], fin[:, 384:D],
                             start=True, stop=True)
            sqA = tailp.tile([1, 384], F32)
            sqB = tailp.tile([1, 384], F32)
            negA = tailp.tile([1, 1], F32)
            negB = tailp.tile([1, 1], F32)
            nc.scalar.activation(sqA[:], sA[:], AF.Square, accum_out=negA[:])
            nc.scalar.activation(sqB[:], sB[:], AF.Square, accum_out=negB[:])

            res = tailp.tile([1, 1], F32)
            nc.vector.tensor_add(res[:], negA[:], negB[:])
            nc.vector.tensor_sub(res[:], res[:], pos_sb[:])
            nc.vector.tensor_scalar(res[:], res[:], 1.0 / D, None, OP.mult)
            nc.sync.dma_start(out=out_d.ap(), in_=res[:])

    nc.finalize()
    return nc


_NC_CACHE = {}


def _get_nc(trivial_wb):
    if trivial_wb not in _NC_CACHE:
        _NC_CACHE[trivial_wb] = build_kernel(trivial_wb=trivial_wb)
    return _NC_CACHE[trivial_wb]


def make_in_maps(input_f, char_dic, ln_w, ln_b, target):
    input_f = np.ascontiguousarray(np.asarray(input_f, dtype=np.float32))
    char_dic = np.ascontiguousarray(np.asarray(char_dic, dtype=np.float32))
    ln_w = np.asarray(ln_w, dtype=np.float32)
    ln_b = np.asarray(ln_b, dtype=np.float32)
    labels = np.asarray(target).reshape(B, S).astype(np.float32)

    wbc = np.ascontiguousarray(np.broadcast_to(ln_w[None, :], (NCHAR, D)))
    bbc = np.ascontiguousarray(np.broadcast_to(ln_b[None, :], (NCHAR, D)))

    bpc = B // N_CORES
    in_maps = []
    for c in range(N_CORES):
        x_c = input_f[c * bpc:(c + 1) * bpc].reshape(TOK_PER_CORE, D)
        l_c = labels[c * bpc:(c + 1) * bpc].reshape(TOK_PER_CORE)
        in_maps.append({
            "x": np.ascontiguousarray(x_c),
            "lab": np.ascontiguousarray(l_c),
            "char": char_dic,
            "wbc": wbc,
            "bbc": bbc,
        })
    return in_maps


def run(trace=False, **inputs):
    trivial_wb = bool(
        np.all(np.asarray(inputs["ln_w"], dtype=np.float32) == 1.0)
        and np.all(np.asarray(inputs["ln_b"], dtype=np.float32) == 0.0))
    nc = _get_nc(trivial_wb)
    in_maps = make_in_maps(**inputs)
    res = run_bass_kernel_spmd(nc, in_maps, core_ids=list(range(N_CORES)),
                               trace=trace)
    out = np.float32(res.results[0]["out"][0, 0])
    return out, res


def kernel(**inputs):
    out, _ = run(trace=False, **inputs)
    return np.array(out, dtype=np.float32)


if __name__ == "__main__":
    np.random.seed(0)
    input_f = np.random.randn(B, S, D).astype(np.float32)
    char_dic = np.random.randn(NCHAR, D).astype(np.float32)
    ln_w = np.ones(D, np.float32)
    ln_b = np.zeros(D, np.float32)
    target = np.random.randint(0, NCHAR, (B, S)).astype(np.int64)
    out = kernel(input_f=input_f, char_dic=char_dic, ln_w=ln_w,
                 ln_b=ln_b, target=target)
    print("kernel out:", out)
